# revision 51
# baseline (speedup 1.0000x reference)
"""CosineAttention on 8 TRN2 NeuronCores — v3.

Sharding: core c owns batch g=c//4 and heads {2r, 2r+1} (r=c%4), stacked on
the 128-partition axis.
  - sim: K=64 matmuls 2-way row-tiled (tile_position (0,0)/(64,0)) so both
    heads' sims run concurrently in the PE array
  - exp at [128, 1024] granularity (both heads per instr) straight from PSUM;
    single ACT table set (ln/exp/square/copy) for the whole kernel
  - attn@v with a ones column (softmax denominator = row 64); 1/sumexp via
    DVE reciprocal_approx_fast (SBUF input only!)
  - free-axis broadcasts (rms scale, 1/sumexp) via SBUF->SBUF DMA with a
    0-stride partition source instead of PE outer products
  - per-512-token-chunk AllGather over the 4 cores of the batch group;
    out-proj for chunk g is issued after attention of chunk g+1 so the FIFO
    PE queue never stalls on the collective
  - stage-1 normalization of chunk c issues after chunk c+1's projections
    (PE never waits on the ACT chain)
"""

import numpy as np
import ml_dtypes

import concourse.bass as bass
import concourse.tile as tile
from concourse import bacc
import concourse.mybir as mybir
from concourse import bass_utils

f32 = mybir.dt.float32
f32r = mybir.dt.float32r
bf16 = mybir.dt.bfloat16
AF = mybir.ActivationFunctionType

N_CORES = 8
HEADS = 8
D = 64            # head dim
B = 2             # batch
SEQ = 2048        # tokens per batch (= tokens per core)
DIM = 512         # model dim
SCALE = D ** -0.5  # 0.125

FT = DIM // 128   # 4 k-tiles of 128
CH = 512          # stage-1 token chunk
NCH = SEQ // CH               # 4
JT = SEQ // 128   # 16 j-tiles
IH = 512          # attention i-chunk (= collective chunk)
NIH = SEQ // IH               # 4

_BUILD_CACHE = {}


def build(collective=True, num_devices=N_CORES, dbg=False):
    key = (collective, num_devices, dbg)
    if key in _BUILD_CACHE:
        return _BUILD_CACHE[key]
    nc = bacc.Bacc("TRN2", target_bir_lowering=False, debug=False,
                   num_devices=num_devices)
    xT = nc.dram_tensor("xT", [DIM, SEQ], bf16, kind="ExternalInput").ap()
    wq = nc.dram_tensor("wq", [128, FT * 128], bf16, kind="ExternalInput").ap()
    wk = nc.dram_tensor("wk", [128, FT * 128], bf16, kind="ExternalInput").ap()
    wv = nc.dram_tensor("wv", [128, FT * 128], bf16, kind="ExternalInput").ap()
    w2 = nc.dram_tensor("w2", [128, FT * 128], bf16, kind="ExternalInput").ap()
    o2 = nc.dram_tensor("o2", [128, 2], f32, kind="ExternalInput").ap()
    e2 = nc.dram_tensor("e2", [2, 128], f32, kind="ExternalInput").ap()
    o1 = nc.dram_tensor("o1", [1, 128], f32, kind="ExternalInput").ap()
    outT = nc.dram_tensor("outT", [128, SEQ], f32, kind="ExternalOutput").ap()
    if dbg:
        qnD = nc.dram_tensor("qnD", [128, SEQ], f32, kind="ExternalOutput").ap()
        knD = nc.dram_tensor("knD", [128, SEQ], f32, kind="ExternalOutput").ap()
        ocD = nc.dram_tensor("ocD", [128, IH], f32, kind="ExternalOutput").ap()

    with tile.TileContext(nc) as tc:
        with (
            tc.tile_pool(name="persist", bufs=1) as pp,
            tc.tile_pool(name="sb", bufs=2) as sb,
            tc.tile_pool(name="ps", bufs=1, space="PSUM") as ps,
            tc.tile_pool(name="dram", bufs=1, space="DRAM") as dram,
            nc.allow_low_precision(reason="bf16 matmuls; f32r helpers"),
        ):
            # ---- persistent activations / constants ----
            qn_sb = pp.tile([128, SEQ], bf16)    # rows 0-63 hA, 64-127 hB
            kn_sb = pp.tile([128, SEQ], bf16)
            vo_sb = pp.tile([128, JT, 130], bf16)  # [v_hA|1|v_hB|1] per j-tile
            nc.gpsimd.memset(vo_sb[:], 1.0)

            # weights arrive pre-tiled host-side: [128, FT*128] row-major
            wq_sb = pp.tile([128, FT, 128], bf16)
            wk_sb = pp.tile([128, FT, 128], bf16)
            wv_sb = pp.tile([128, FT, 128], bf16)
            w2_sb = pp.tile([128, FT, 128], bf16)
            for w_sb, w_dr in ((wq_sb, wq), (wk_sb, wk), (wv_sb, wv)):
                nc.sync.dma_start(
                    w_sb[:], w_dr.rearrange("p (t m) -> p t m", t=FT))
            o2_sb = pp.tile([128, 2], f32r)
            nc.sync.dma_start(o2_sb[:], o2[:].bitcast(f32r))
            e2_sb = pp.tile([2, 128], f32r)
            nc.sync.dma_start(e2_sb[:], e2[:].bitcast(f32r))
            o1f_sb = pp.tile([1, 128], f32)
            nc.sync.dma_start(o1f_sb[:], o1[:])
            # w2 only needed at the tail out-projs
            nc.sync.dma_start(
                w2_sb[:], w2.rearrange("p (t m) -> p t m", t=FT))

            # warmup collective: absorbs the first-collective trigger cost
            # and aligns the group before the real chunked AllGathers
            if collective:
                wu_sb = sb.tile([128, 16], bf16, tag="wu")
                nc.gpsimd.memset(wu_sb[:], 0.0)
                wu_in = dram.tile([128, 16], bf16, name="wu_in")
                wu_out = dram.tile([512, 16], bf16, name="wu_out")
                nc.sync.dma_start(wu_in[:], wu_sb[:])
                nc.gpsimd.collective_compute(
                    "AllGather", mybir.AluOpType.bypass,
                    replica_groups=[[0, 1, 2, 3], [4, 5, 6, 7]],
                    ins=[wu_in[:]], outs=[wu_out[:]])

            # ---- stage 1: projections + rms normalization ----
            # Software-pipelined: phase A(c) = dma + q/k/v matmuls + sq;
            # phase B(c) = st matmuls + ln/exp + broadcast + qn/kn/vo.
            stash = {}

            def stage_a(c):
                cols = slice(c * CH, (c + 1) * CH)
                xt = sb.tile([128, FT, CH], bf16, tag="xt", bufs=3)
                for t in range(FT):
                    nc.sync.dma_start(xt[:, t, :],
                                      xT[t * 128:(t + 1) * 128, cols])
                qk_ps = ps.tile([128, 2, CH], f32, tag="sim", bufs=2)
                for t in range(FT):
                    nc.tensor.matmul(qk_ps[:, 0, :], wq_sb[:, t, :],
                                     xt[:, t, :],
                                     start=(t == 0), stop=(t == FT - 1))
                for t in range(FT):
                    nc.tensor.matmul(qk_ps[:, 1, :], wk_sb[:, t, :],
                                     xt[:, t, :],
                                     start=(t == 0), stop=(t == FT - 1))
                vb_ps = ps.tile([128, CH], f32, tag="acc", bufs=3)
                for js in range(CH // 128):
                    for t in range(FT):
                        nc.tensor.matmul(
                            vb_ps[:, js * 128:(js + 1) * 128],
                            xt[:, t, js * 128:(js + 1) * 128],
                            wv_sb[:, t, :],
                            start=(t == 0), stop=(t == FT - 1))
                sq = sb.tile([128, 2, CH], f32r, tag="sq", bufs=3)
                nc.scalar.activation(sq[:], qk_ps[:], AF.Square)
                # v -> vo here so vb_ps drains within phase A
                for js in range(CH // 128):
                    jt = c * (CH // 128) + js
                    nc.vector.tensor_copy(
                        vo_sb[:, jt, 0:64],
                        vb_ps[:, js * 128:js * 128 + 64])
                    nc.vector.tensor_copy(
                        vo_sb[:, jt, 65:129],
                        vb_ps[:, js * 128 + 64:(js + 1) * 128])
                stash[c] = (qk_ps, sq)

            def stage_b(c):
                cols = slice(c * CH, (c + 1) * CH)
                qk_ps, sq = stash.pop(c)
                st_q = ps.tile([128, CH], f32, tag="acc", bufs=3)
                st_k = ps.tile([128, CH], f32, tag="acc", bufs=3)
                nc.tensor.matmul(st_q[0:2, :], o2_sb[:], sq[:, 0, :],
                                 start=True, stop=True)
                nc.tensor.matmul(st_k[0:2, :], o2_sb[:], sq[:, 1, :],
                                 start=True, stop=True)
                # r = (st/64)^-0.5 via ln+exp (one ACT table set kernel-wide)
                lt = sb.tile([2, 2, CH], f32, tag="lt")
                nc.scalar.activation(lt[:, 0, :], st_q[0:2, :], AF.Ln,
                                     scale=1.0 / D)
                nc.scalar.activation(lt[:, 1, :], st_k[0:2, :], AF.Ln,
                                     scale=1.0 / D)
                r4 = sb.tile([2, 2, CH], f32r, tag="r4")
                nc.scalar.activation(r4[:], lt[:], AF.Exp, scale=-0.5)
                # broadcast r along the 64 rows of each head (PE outer
                # product, one bank reused q then k)
                rb = sb.tile([128, 2, CH], f32, tag="rb")
                for j in range(2):          # q | k
                    rb_ps = ps.tile([128, CH], f32, tag="rbp", bufs=1)
                    nc.tensor.matmul(rb_ps[:], e2_sb[:], r4[:, j, :],
                                     start=True, stop=True)
                    nc.vector.tensor_copy(rb[:, j, :], rb_ps[:])
                nc.vector.tensor_mul(qn_sb[:, cols], qk_ps[:, 0, :],
                                     rb[:, 0, :])
                nc.vector.tensor_mul(kn_sb[:, cols], qk_ps[:, 1, :],
                                     rb[:, 1, :])

            stage_a(0)
            for c in range(1, NCH):
                stage_a(c)
                stage_b(c - 1)
            stage_b(NCH - 1)

            if dbg:
                for c in range(NCH):
                    cols = slice(c * CH, (c + 1) * CH)
                    for src, dst in ((qn_sb, qnD), (kn_sb, knD)):
                        dt_ = sb.tile([128, CH], f32, tag="dbg")
                        nc.vector.tensor_copy(dt_[:], src[:, cols])
                        nc.sync.dma_start(dst[:, cols], dt_[:])

            # ---- attention + chunked collective + deferred out-proj ----
            pend = {}

            def attention(g, finalize):
                i0 = g * IH
                av_a = ps.tile([128, IH], f32, tag="acc", bufs=3)
                av_b = ps.tile([128, IH], f32, tag="acc", bufs=3)
                for jt in range(JT):
                    jc = slice(jt * 128, (jt + 1) * 128)
                    simp = ps.tile([128, 2, IH], f32, tag="sim", bufs=2)
                    nc.tensor.matmul(simp[:, 0, :], kn_sb[0:64, jc],
                                     qn_sb[0:64, i0:i0 + IH],
                                     start=True, stop=True,
                                     tile_position=(0, 0))
                    nc.tensor.matmul(simp[:, 1, :], kn_sb[64:128, jc],
                                     qn_sb[64:128, i0:i0 + IH],
                                     start=True, stop=True,
                                     tile_position=(64, 0))
                    expt = sb.tile([128, 2, IH], bf16, tag="expt", bufs=3)
                    nc.scalar.activation(expt[:], simp[:], AF.Exp, scale=SCALE)
                    nc.tensor.matmul(av_a[0:65, :], vo_sb[:, jt, 0:65],
                                     expt[:, 0, :],
                                     start=(jt == 0), stop=(jt == JT - 1),
                                     skip_group_check=True)
                    nc.tensor.matmul(av_b[0:65, :], vo_sb[:, jt, 65:130],
                                     expt[:, 1, :],
                                     start=(jt == 0), stop=(jt == JT - 1),
                                     skip_group_check=True)
                    if jt == 0:
                        # start the previous chunk's 1/sumexp chain on DVE
                        # while this chunk's first sims run on PE
                        if g - 1 in pend:
                            finalize_rsec(g - 1)
                    if jt == 2:
                        if g - 1 in pend:
                            finalize(g - 1)
                pend[g] = (av_a, av_b)

            def finalize_rsec(g):
                av_a, av_b = pend[g]
                se_sb = sb.tile([1, 2, IH], f32, tag="se")
                nc.vector.tensor_copy(se_sb[0:1, 0, :], av_a[64:65, :])
                nc.vector.tensor_copy(se_sb[0:1, 1, :], av_b[64:65, :])
                rsec = sb.tile([1, 2, IH], f32, tag="rsec")
                nc.vector.reciprocal_approx_fast(rsec[0:1, 0, :],
                                                 se_sb[0:1, 0, :])
                nc.vector.reciprocal_approx_fast(rsec[0:1, 1, :],
                                                 se_sb[0:1, 1, :])
                pend[g] = (av_a, av_b, rsec)

            def finalize(g):
                av_a, av_b, rsec = pend.pop(g)
                cc_in = dram.tile([128, IH], bf16, name=f"cc_in{g}")
                cc_out = dram.tile([DIM, IH], bf16, name=f"cc_out{g}")
                r2_ps = ps.tile([128, IH], f32, tag="rbp", bufs=1)
                nc.tensor.matmul(r2_ps[0:64, :], o1f_sb[0:1, 0:64],
                                 rsec[0:1, 0, :], start=True, stop=True)
                nc.tensor.matmul(r2_ps[64:128, :], o1f_sb[0:1, 64:128],
                                 rsec[0:1, 1, :], start=True, stop=True,
                                 tile_position=(0, 64))
                r2 = sb.tile([128, IH], f32, tag="r2")
                nc.vector.tensor_copy(r2[:], r2_ps[:])
                occ = sb.tile([128, IH], bf16, tag="occ")
                nc.vector.tensor_mul(occ[0:64, :], av_a[0:64, :], r2[0:64, :])
                nc.vector.tensor_mul(occ[64:128, :], av_b[0:64, :],
                                     r2[64:128, :])
                if dbg and g == 0:
                    dt_ = sb.tile([128, IH], f32, tag="dbgo")
                    nc.vector.tensor_copy(dt_[:], occ[:])
                    nc.sync.dma_start(ocD[:], dt_[:])
                nc.sync.dma_start(cc_in[:], occ[:])
                if collective:
                    nc.gpsimd.collective_compute(
                        "AllGather", mybir.AluOpType.bypass,
                        replica_groups=[[0, 1, 2, 3], [4, 5, 6, 7]],
                        ins=[cc_in[:]], outs=[cc_out[:]])
                else:
                    nc.sync.dma_start(cc_out[0:128, :], cc_in[:])
                return cc_out

            def out_proj(g, cc_out):
                i0 = g * IH
                ag = sb.tile([128, FT, IH], bf16, tag="ag")
                for t in range(FT):
                    nc.sync.dma_start(ag[:, t, :],
                                      cc_out[t * 128:(t + 1) * 128, :])
                op_ps = ps.tile([128, IH], f32, tag="acc", bufs=3)
                for t in range(FT):
                    nc.tensor.matmul(op_ps[:], w2_sb[:, t, :], ag[:, t, :],
                                     start=(t == 0), stop=(t == FT - 1))
                fo = sb.tile([128, IH], f32, tag="fo")
                nc.vector.tensor_copy(fo[:], op_ps[:])
                nc.sync.dma_start(outT[:, i0:i0 + IH], fo[:])

            ccs = {}
            for g in range(NIH):
                attention(g, lambda gg: ccs.__setitem__(gg, finalize(gg)))
            finalize_rsec(NIH - 1)
            ccs[NIH - 1] = finalize(NIH - 1)
            for g in range(NIH):
                out_proj(g, ccs[g])
    nc.compile()
    _BUILD_CACHE[key] = nc
    return nc


def make_in_maps(x, Wq, Wkv, Wout):
    o2 = np.zeros((128, 2), np.float32)
    o2[0:64, 0] = 1.0
    o2[64:128, 1] = 1.0
    e2 = np.zeros((2, 128), np.float32)
    e2[0, 0:64] = 1.0
    e2[1, 64:128] = 1.0
    o1 = np.ones((1, 128), np.float32)
    bf = ml_dtypes.bfloat16
    in_maps = []
    for c in range(N_CORES):
        g, r = c // 4, c % 4
        hrows = slice(2 * r * D, (2 * r + 2) * D)
        xTc = np.ascontiguousarray(x[g].T).astype(bf)
        def tile_w(w):
            # [DIM, 128] -> [128, FT*128]: row p holds [t0 cols | t1 | ...]
            return np.ascontiguousarray(
                w.reshape(FT, 128, 128).transpose(1, 0, 2).reshape(128, -1)
            ).astype(bf)
        wqc = tile_w(Wq[hrows, :].T)
        wkc = tile_w(Wkv[hrows, :].T)
        wvc = tile_w(Wkv[DIM + 2 * r * D:DIM + (2 * r + 2) * D, :].T)
        w2c = tile_w(Wout[128 * r:128 * (r + 1), :].T)
        in_maps.append({
            "xT": xTc, "wq": wqc, "wk": wkc, "wv": wvc, "w2": w2c,
            "o2": o2, "e2": e2, "o1": o1,
        })
    return in_maps


def kernel(x, Wq, Wkv, Wout, _trace=False):
    nc = build()
    in_maps = make_in_maps(np.asarray(x), np.asarray(Wq), np.asarray(Wkv),
                           np.asarray(Wout))
    res = bass_utils.run_bass_kernel_spmd(
        nc, in_maps, core_ids=list(range(N_CORES)), trace=_trace)
    full = np.empty((B, SEQ, DIM), np.float32)
    for c in range(N_CORES):
        g, r = c // 4, c % 4
        full[g, :, 128 * r:128 * (r + 1)] = res.results[c]["outT"].T
    if _trace:
        return full, res
    return full
